# revision 1
# baseline (speedup 1.0000x reference)
"""Trainium2 Bass kernel: dense-masked sliding-window attention.

nn.Module semantics (see harness reference):
    B,S,E,H,W = 1, 4096, 1024, 16, 256; D = 64
    qkv = x @ w_qkv -> q,k,v  [B,S,H,D]
    scores = q k^T / sqrt(D), masked to the sliding causal window
             (key j allowed for query i iff i-W < j <= i)
    out = softmax(scores) v  -> [B,S,E] @ w_out

Sharding: sequence-parallel over 8 NeuronCores. Core c owns queries
[512c, 512c+512) and receives a 256-row key/value halo on the left; no
collectives are needed (host concatenates the per-core output rows).

Per-core kernel layout choices:
  - x is shipped pre-transposed ([E, 768] feature-major) so every matmul
    contracts over the partition dim without on-device transposes.
  - q^T, k^T are produced transposed ([dims, seq]); v natural
    ([seq, dims]), stored interleaved with a ones column per head so the
    attention row-sums (softmax denominators) fall out of the attention
    value accumulation as an extra output row.
  - scores are built transposed ([t, s]) in key-chunk-major band groups;
    softmax skips the max-subtraction (scores are O(1) here: exp cannot
    overflow) so exp is one activation op per head; the window/causal
    mask is applied multiplicatively on exp(scores) with mask data taken
    from the allowed_mask input (any mask inside the band is honored).
  - all matmuls run in fp16 (full PE rate at any moving width; fp32r
    drops to 1/4 rate below N=256). End-to-end error vs the fp32
    reference is ~4e-4 scale-relative.
"""

import numpy as np
from contextlib import ExitStack

import concourse.bass as bass
import concourse.tile as tile
from concourse import bacc, mybir
from concourse.bass_utils import run_bass_kernel_spmd

F32 = mybir.dt.float32
F32R = mybir.dt.float32r
F16 = mybir.dt.float16

B, S, E, H, W = 1, 4096, 1024, 16, 256
D = E // H  # 64
SCALE = D ** -0.5
N_CORES = 8
SQ = S // N_CORES          # 512 queries per core
HALO = W                   # 256 halo keys
SK = SQ + HALO             # 768 key rows per core
KC = E // 128              # 8 contraction chunks
QB = SQ // 128             # 4 query blocks per core
TC = SK // 128             # 6 key chunks per core
VW = H * (D + 1)           # 1040: v row width with ones columns

# band groups, key-chunk major: for key chunk T the valid query blocks
# are qb in [T-2, T] (clipped); groups are contiguous in both the scores
# free dim and the query dim.  Group order is chosen so no group crosses
# a 512-column PSUM bank boundary.
_GORDER = [2, 0, 3, 5, 1, 4]
GRP = []  # (T, qb0, nqb, col0)
_c = 0
for _T in _GORDER:
    _q0 = max(0, _T - 2)
    _qe = min(QB - 1, _T)
    GRP.append((_T, _q0, _qe - _q0 + 1, _c))
    _c += (_qe - _q0 + 1) * 128
NSLICE_COLS = _c  # 1536


def _bank_split(col0, width):
    """Split a [col0, col0+width) span at 512-col PSUM bank boundaries."""
    out = []
    c = col0
    while c < col0 + width:
        e = min(col0 + width, (c // 512 + 1) * 512)
        out.append((c, e - c))
        c = e
    return out


def _emit_body(ctx: ExitStack, tc_: "tile.TileContext", xT_d, wq_d, wk_d, wv_d,
               wout_d, pmask_d, out_d):
    nc = tc_.nc
    P = 128

    xt_pool = ctx.enter_context(tc_.tile_pool(name="xt", bufs=KC))
    w_pool = ctx.enter_context(tc_.tile_pool(name="w", bufs=10))
    qt_pool = ctx.enter_context(tc_.tile_pool(name="qt", bufs=KC))
    kt_pool = ctx.enter_context(tc_.tile_pool(name="kt", bufs=KC))
    v_pool = ctx.enter_context(tc_.tile_pool(name="v", bufs=TC))
    pm_pool = ctx.enter_context(tc_.tile_pool(name="pm", bufs=1))
    et_pool = ctx.enter_context(tc_.tile_pool(name="et", bufs=3))
    at_pool = ctx.enter_context(tc_.tile_pool(name="at", bufs=KC))
    os_pool = ctx.enter_context(tc_.tile_pool(name="os", bufs=2))
    nrm_pool = ctx.enter_context(tc_.tile_pool(name="nrm", bufs=4))
    ps_big = ctx.enter_context(tc_.tile_pool(name="psb", bufs=2, space="PSUM"))
    ps_ot = ctx.enter_context(tc_.tile_pool(name="pso", bufs=2, space="PSUM"))

    # ---- loads (wq/xt first: they gate the first matmuls) ----------------
    wq, xt = [], []
    for k in range(KC):
        t = w_pool.tile([P, 1024], F16, tag="w")
        nc.sync.dma_start(t[:], wq_d.ap()[k * P:(k + 1) * P, :])
        wq.append(t)
        t = xt_pool.tile([P, SK], F16, tag="xt")
        nc.sync.dma_start(t[:], xT_d.ap()[k * P:(k + 1) * P, :])
        xt.append(t)

    # ---- q^T [E, SQ]: stationary = w_q chunk columns, moving = x^T -------
    qt = []
    for n in range(KC):
        if n % 2 == 0:
            ps_full = ps_big.tile([P, 1536], F32, tag="psb", name=f"qtp{n}")
            ps = ps_full[:, 0:SQ]
        else:
            ps = ps_ot.tile([P, SQ], F32, tag="pso", name=f"qtp{n}")
        for k in range(KC):
            nc.tensor.matmul(ps[:, 0:SQ], wq[k][:, n * P:(n + 1) * P],
                             xt[k][:, HALO:SK], start=(k == 0), stop=(k == KC - 1))
        t = qt_pool.tile([P, SQ], F16, tag="qt")
        nc.scalar.copy(t[:], ps[:, 0:SQ])
        qt.append(t)

    # ---- k^T [E, SK] -----------------------------------------------------
    wk = []
    for k in range(KC):
        t = w_pool.tile([P, 1024], F16, tag="w")
        nc.sync.dma_start(t[:], wk_d.ap()[k * P:(k + 1) * P, :])
        wk.append(t)
    kt = []
    for n in range(KC):
        ps = ps_big.tile([P, 1536], F32, tag="psb")
        for k in range(KC):
            nc.tensor.matmul(ps[:, 0:512], wk[k][:, n * P:(n + 1) * P],
                             xt[k][:, 0:512], start=(k == 0), stop=(k == KC - 1))
            nc.tensor.matmul(ps[:, 512:768], wk[k][:, n * P:(n + 1) * P],
                             xt[k][:, 512:768], start=(k == 0), stop=(k == KC - 1))
        t = kt_pool.tile([P, SK], F16, tag="kt")
        nc.scalar.copy(t[:], ps[:, 0:SK])
        kt.append(t)

    # ---- v natural [SK, 16*(64+1)] --------------------------------------
    # w_v is shipped host-interleaved [E, 1040] with a zero column after
    # each head's 64 dims; the zero columns become the ones columns.
    wv = []
    for k in range(KC):
        t = w_pool.tile([P, 1024], F16, tag="wv")
        nc.sync.dma_start(t[:], wv_d.ap()[k * P:(k + 1) * P, :])
        wv.append(t)
    ones_f = nrm_pool.tile([P, 1], F32, tag="ones")
    nc.vector.memset(ones_f[:], 1.0)
    vt = []
    for sc in range(TC):
        ps = ps_big.tile([P, 1536], F32, tag="psb")
        for k in range(KC):
            nc.tensor.matmul(ps[:, 0:512], xt[k][:, sc * P:(sc + 1) * P],
                             wv[k][:, 0:512], start=(k == 0), stop=(k == KC - 1))
            nc.tensor.matmul(ps[:, 512:1024], xt[k][:, sc * P:(sc + 1) * P],
                             wv[k][:, 512:1024], start=(k == 0), stop=(k == KC - 1))
        t = v_pool.tile([P, VW], F16, tag="v")
        nc.scalar.copy(
            t[:].rearrange("p (h c) -> p h c", h=H)[:, :, 0:D],
            ps[:, 0:1024].rearrange("p (h c) -> p h c", h=H))
        nc.vector.tensor_copy(
            t[:].rearrange("p (h c) -> p h c", h=H)[:, :, D:D + 1],
            ones_f[:, None, :].broadcast_to([P, H, 1]))
        vt.append(t)

    pm = pm_pool.tile([P, NSLICE_COLS], F16)
    nc.sync.dma_start(pm[:], pmask_d.ap()[:])

    # ---- attention (heads paired on PE row halves) -----------------------
    at = []
    for p in range(KC):  # 8 head pairs / e-chunks
        a = at_pool.tile([P, SQ], F16, tag="at")
        at.append(a)
        # QK for both heads of the pair back-to-back: the row-tiled
        # matmuls (rows 0-63 vs 64-127) run concurrently on the PE.
        sc_ps = []
        for sub in range(2):
            r0 = 64 * sub
            sp = ps_big.tile([P, 1536], F32, tag="psb")
            sc_ps.append(sp)
            for (T, q0, nq, col0) in GRP:
                nc.tensor.matmul(
                    sp[:, col0:col0 + nq * 128],
                    kt[p][r0:r0 + 64, T * P:(T + 1) * P],
                    qt[p][r0:r0 + 64, q0 * 128:(q0 + nq) * 128],
                    start=True, stop=True, tile_position=(r0, 0))
        for sub in range(2):
            h = 2 * p + sub
            r0 = 64 * sub
            et = et_pool.tile([P, 1536], F16, tag="et")
            for b0 in range(0, 1536, 512):
                nc.scalar.activation(et[:, b0:b0 + 512], sc_ps[sub][:, b0:b0 + 512],
                                     mybir.ActivationFunctionType.Exp)
                nc.vector.tensor_tensor(et[:, b0:b0 + 512], et[:, b0:b0 + 512],
                                        pm[:, b0:b0 + 512], mybir.AluOpType.mult)
            # attention-value products; denominators land in row 64
            ot = ps_ot.tile([P, SQ], F32, tag="pso")
            for gi, (T, q0, nq, col0) in enumerate(GRP):
                nc.tensor.matmul(
                    ot[0:65, q0 * 128:(q0 + nq) * 128],
                    vt[T][:, h * 65:h * 65 + 65],
                    et[:, col0:col0 + nq * 128],
                    start=(gi == 0), stop=(gi == len(GRP) - 1))
            # normalize rows by the per-query denominator
            rc = nrm_pool.tile([1, SQ], F32, tag="rc")
            nc.vector.reciprocal(rc[:], ot[64:65, :])
            rb = nrm_pool.tile([64, SQ], F32, tag="rb")
            nc.gpsimd.partition_broadcast(rb[:], rc[:])
            nc.vector.tensor_tensor(a[r0:r0 + 64, :], ot[0:64, :], rb[:],
                                    mybir.AluOpType.mult)

    # ---- output projection ----------------------------------------------
    wo = []
    for p in range(KC):
        t = w_pool.tile([P, 1024], F16, tag="w")
        nc.sync.dma_start(t[:], wout_d.ap()[p * P:(p + 1) * P, :])
        wo.append(t)
    for sb in range(QB):
        ps = ps_big.tile([P, 1536], F32, tag="psb")
        for p in range(KC):
            nc.tensor.matmul(ps[:, 0:512], at[p][:, sb * P:(sb + 1) * P],
                             wo[p][:, 0:512], start=(p == 0), stop=(p == KC - 1))
            nc.tensor.matmul(ps[:, 512:1024], at[p][:, sb * P:(sb + 1) * P],
                             wo[p][:, 512:1024], start=(p == 0), stop=(p == KC - 1))
        ob = os_pool.tile([P, E], F32, tag="os")
        nc.scalar.copy(ob[:], ps[:, 0:E])
        nc.sync.dma_start(out_d.ap()[sb * P:(sb + 1) * P, :], ob[:])


def build(n_iters: int = 1):
    nc = bacc.Bacc("TRN2", target_bir_lowering=False, debug=False,
                   num_devices=N_CORES)
    xT_d = nc.dram_tensor("xT", [E, SK], F16, kind="ExternalInput")
    wq_d = nc.dram_tensor("wq", [E, E], F16, kind="ExternalInput")
    wk_d = nc.dram_tensor("wk", [E, E], F16, kind="ExternalInput")
    wv_d = nc.dram_tensor("wv", [E, E], F16, kind="ExternalInput")
    wout_d = nc.dram_tensor("wout", [E, E], F16, kind="ExternalInput")
    pmask_d = nc.dram_tensor("pmask", [128, NSLICE_COLS], F16,
                             kind="ExternalInput")
    out_d = nc.dram_tensor("out", [SQ, E], F32, kind="ExternalOutput")
    with tile.TileContext(nc) as tc_, ExitStack() as ctx:
        if n_iters > 1:
            with tc_.For_i(0, n_iters, 1):
                _emit_body(ctx, tc_, xT_d, wq_d, wk_d, wv_d, wout_d, pmask_d,
                           out_d)
        else:
            _emit_body(ctx, tc_, xT_d, wq_d, wk_d, wv_d, wout_d, pmask_d,
                       out_d)
    nc.compile()
    return nc


def make_in_maps(x, allowed_mask, w_qkv, w_out):
    """Shard the full inputs into per-core input maps (host marshaling)."""
    x2 = np.asarray(x, dtype=np.float32).reshape(S, E)
    wqkv = np.asarray(w_qkv, dtype=np.float32)
    wq = np.ascontiguousarray(wqkv[:, 0:E]) * np.float32(SCALE)
    wk = np.ascontiguousarray(wqkv[:, E:2 * E])
    wv = np.ascontiguousarray(wqkv[:, 2 * E:3 * E])
    wout = np.ascontiguousarray(np.asarray(w_out, dtype=np.float32))
    am = np.asarray(allowed_mask).reshape(S, S)

    xT = np.ascontiguousarray(x2.T)  # [E, S]
    in_maps = []
    for c in range(N_CORES):
        lo = c * SQ - HALO
        xTc = np.zeros((E, SK), dtype=np.float32)
        ofs = max(0, -lo)
        xTc[:, ofs:] = xT[:, lo + ofs:c * SQ + SQ]
        pmask = np.zeros((128, NSLICE_COLS), dtype=np.float32)
        for (T, q0, nq, col0) in GRP:
            t0 = lo + T * 128
            if t0 + 128 <= 0:
                continue
            tlo = max(0, -t0)
            s0 = c * SQ + q0 * 128
            blk = am[s0:s0 + nq * 128, t0 + tlo:t0 + 128]  # [s, t]
            pmask[tlo:128, col0:col0 + nq * 128] = blk.T.astype(np.float32)
        in_maps.append({
            "xT": xTc.astype(np.float16),
            "wq": wq.astype(np.float16),
            "wk": wk.astype(np.float16),
            "wv": wv.astype(np.float16),
            "wout": wout.astype(np.float16),
            "pmask": pmask.astype(np.float16),
        })
    return in_maps


_CACHED_NC = None


def kernel(x, allowed_mask, w_qkv, w_out):
    global _CACHED_NC
    if _CACHED_NC is None:
        _CACHED_NC = build()
    in_maps = make_in_maps(x, allowed_mask, w_qkv, w_out)
    res = run_bass_kernel_spmd(_CACHED_NC, in_maps, list(range(N_CORES)))
    out = np.concatenate([res.results[c]["out"] for c in range(N_CORES)], axis=0)
    return out.reshape(B, S, E)

